# revision 5
# baseline (speedup 1.0000x reference)
"""Distributed Trainium2 kernel for the DPCE loss.

loss = -mean_{b,p}[ sum_c dist_y[b,c,p] * logp[b,c,p] ]

where dist_y[:,0] = onehot0, dist_y[:,i>=1] = (z_i - mn_i)/(mx_i + eps - mn_i),
z_i = onehot_i * dist, mn/mx per (b, i) over all spatial positions, and
logp = log_softmax(net_output, axis=1).

Factorization (per batch b, class i in 1..3):
    sum_p zn_i * logp_i = (A_i - mn_i * L_i) / (mx_i + eps - mn_i)
with A_i = sum_p z_i * logp_i,  L_i = sum_p logp_i,  plus the class-0 term
E = sum_p onehot0 * logp_0.  All of A/L/E/mx (and class counts, used to
resolve mn exactly) are single-pass masked reductions -> fully data-parallel
over the depth axis across 8 cores, with a tiny host-side combine.

mn_i = min_p z_i is exactly 0 unless EVERY position of batch b has class i
(z_i >= 0 with zeros wherever target != i); that case is detected via the
exact class counts and resolved on the host from the f32 dist tensor.
"""

import os
from contextlib import ExitStack

import numpy as np
import ml_dtypes

import concourse.bass as bass
import concourse.tile as tile
from concourse import bacc, mybir
from concourse.bass_utils import run_bass_kernel_spmd

# Problem shape (hardcoded per the task contract).
B, C, D, H, W = 2, 4, 128, 192, 192
NCORES = 8
DSH = D // NCORES            # depth slices per core
P = 128                      # SBUF partitions
SP = DSH * H * W             # spatial elems per (b, ch) per core = 589824
FTOT = SP // P               # free elems per partition = 4608
NCHUNK = 2
F = FTOT // NCHUNK           # chunk free size = 2304
EPS = 1e-8

# stats tile column layout, per (b, chunk) group of NSTAT columns:
#   [0:4]   L_c  = sum(x_c - lse)              (c = 0..3; c=0 unused)
#   [4:7]   mn_i = min((target == i) * dist)   (i = 1..3)
#   [7:10]  mx_i = max((target == i) * dist)
#   [10:13] A_i  = sum(w_i * lp_i)
#   [13]    E    = sum((target == 0) * lp_0)
NSTAT = 14
NGRP = B * NCHUNK
NCOL = 64
assert NGRP * NSTAT <= NCOL

_BF = ml_dtypes.bfloat16

_compiled_nc = None


def _build():
    nc = bacc.Bacc("TRN2", target_bir_lowering=False, debug=False)
    bf = mybir.dt.bfloat16
    f32 = mybir.dt.float32
    AF = mybir.ActivationFunctionType
    Op = mybir.AluOpType

    x = nc.dram_tensor("x", [B, C, P, FTOT], bf, kind="ExternalInput").ap()
    t = nc.dram_tensor("t", [B, P, FTOT], bf, kind="ExternalInput").ap()
    d = nc.dram_tensor("d", [B, P, FTOT], bf, kind="ExternalInput").ap()
    out = nc.dram_tensor("out", [P, NCOL], f32, kind="ExternalOutput").ap()

    with tile.TileContext(nc) as tc, ExitStack() as ctx:
        inp = ctx.enter_context(tc.tile_pool(name="inp", bufs=2))
        work = ctx.enter_context(tc.tile_pool(name="work", bufs=2))
        spool = ctx.enter_context(tc.tile_pool(name="stats", bufs=1))

        stats = spool.tile([P, NCOL], f32)
        nc.gpsimd.memset(stats[:], 0.0)

        g = 0
        for b in range(B):
            for ck in range(NCHUNK):
                sl = slice(ck * F, (ck + 1) * F)
                xs = []
                for c in range(C):
                    xt = inp.tile([P, F], bf, tag=f"x{c}")
                    nc.sync.dma_start(xt[:], x[b, c, :, sl])
                    xs.append(xt)
                tt = inp.tile([P, F], bf, tag="t")
                nc.sync.dma_start(tt[:], t[b, :, sl])
                dd = inp.tile([P, F], bf, tag="d")
                nc.sync.dma_start(dd[:], d[b, :, sl])

                # lse = ln(sum_c exp(x_c)); |x| <= ~6 so no max-subtraction
                # is needed at f32 internal precision.
                es = []
                for c in range(C):
                    e = work.tile([P, F], bf, tag=f"e{c}")
                    nc.scalar.activation(e[:], xs[c][:], AF.Exp)
                    es.append(e)
                s01 = work.tile([P, F], bf, tag="s01")
                nc.gpsimd.tensor_tensor(s01[:], es[0][:], es[1][:], op=Op.add)
                s23 = work.tile([P, F], bf, tag="s23")
                nc.gpsimd.tensor_tensor(s23[:], es[2][:], es[3][:], op=Op.add)
                esum = work.tile([P, F], bf, tag="esum")
                nc.gpsimd.tensor_tensor(esum[:], s01[:], s23[:], op=Op.add)
                lse = work.tile([P, F], bf, tag="lse")
                nc.scalar.activation(lse[:], esum[:], AF.Ln)

                col = g * NSTAT
                for c in range(C):
                    # lp_c = x_c - lse, with fused L_c = sum(lp_c)
                    lp = work.tile([P, F], bf, tag="lp")
                    nc.vector.scalar_tensor_tensor(
                        out=lp[:], in0=xs[c][:], scalar=0.0, in1=lse[:],
                        op0=Op.bypass, op1=Op.subtract,
                        accum_out=stats[:, col + c : col + c + 1],
                    )
                    scr = work.tile([P, F], bf, tag="scr")
                    if c == 0:
                        # E = sum((t == 0) * lp_0)
                        nc.vector.scalar_tensor_tensor(
                            out=scr[:], in0=tt[:], scalar=0.0, in1=lp[:],
                            op0=Op.is_equal, op1=Op.mult,
                            accum_out=stats[:, col + 13 : col + 14],
                        )
                    else:
                        # w_i = (t == i) * dist
                        w = work.tile([P, F], bf, tag="w")
                        nc.vector.scalar_tensor_tensor(
                            out=w[:], in0=tt[:], scalar=float(c), in1=dd[:],
                            op0=Op.is_equal, op1=Op.mult,
                        )
                        nc.vector.tensor_reduce(
                            stats[:, col + 4 + (c - 1) : col + 5 + (c - 1)],
                            w[:], axis=mybir.AxisListType.X, op=Op.min,
                        )
                        nc.vector.tensor_reduce(
                            stats[:, col + 7 + (c - 1) : col + 8 + (c - 1)],
                            w[:], axis=mybir.AxisListType.X, op=Op.max,
                        )
                        # A_i = sum(w_i * lp_i)
                        nc.vector.scalar_tensor_tensor(
                            out=scr[:], in0=w[:], scalar=0.0, in1=lp[:],
                            op0=Op.bypass, op1=Op.mult,
                            accum_out=stats[:, col + 10 + (c - 1) : col + 11 + (c - 1)],
                        )
                g += 1

        nc.sync.dma_start(out[:], stats[:])

    nc.compile()
    return nc


def _get_nc():
    global _compiled_nc
    if _compiled_nc is None:
        _compiled_nc = _build()
    return _compiled_nc


def kernel(net_output, target, dist):
    net_output = np.asarray(net_output, dtype=np.float32)
    target = np.asarray(target)
    dist = np.asarray(dist, dtype=np.float32)
    assert net_output.shape == (B, C, D, H, W)

    # host-side prep: bf16 casts + depth sharding
    xb = net_output.astype(_BF).reshape(B, C, NCORES, DSH * H * W)
    tb = target.reshape(B, D, H, W).astype(_BF).reshape(B, NCORES, DSH * H * W)
    db = dist.astype(_BF).reshape(B, NCORES, DSH * H * W)

    in_maps = []
    for r in range(NCORES):
        in_maps.append({
            "x": np.ascontiguousarray(xb[:, :, r]).reshape(B, C, P, FTOT),
            "t": np.ascontiguousarray(tb[:, r]).reshape(B, P, FTOT),
            "d": np.ascontiguousarray(db[:, r]).reshape(B, P, FTOT),
        })

    nc = _get_nc()
    res = run_bass_kernel_spmd(nc, in_maps, core_ids=list(range(NCORES)))

    # host combine (tiny: NCORES * 128 * 64 floats)
    L = np.zeros((B, C))
    mn = np.full((B, C), np.inf)
    mx = np.zeros((B, C))
    A = np.zeros((B, C))
    E = np.zeros(B)
    for r in range(NCORES):
        st = res.results[r]["out"].astype(np.float64)  # [P, NCOL]
        for b in range(B):
            for ck in range(NCHUNK):
                col = (b * NCHUNK + ck) * NSTAT
                L[b] += st[:, col : col + 4].sum(axis=0)
                mn[b, 1:] = np.minimum(mn[b, 1:], st[:, col + 4 : col + 7].min(axis=0))
                mx[b, 1:] = np.maximum(mx[b, 1:], st[:, col + 7 : col + 10].max(axis=0))
                A[b, 1:] += st[:, col + 10 : col + 13].sum(axis=0)
                E[b] += st[:, col + 13].sum()

    n_spatial = D * H * W
    total = 0.0
    for b in range(B):
        acc = E[b]
        for i in range(1, C):
            acc += (A[b, i] - mn[b, i] * L[b, i]) / (mx[b, i] + EPS - mn[b, i])
        total += acc
    loss = -total / (B * n_spatial)
    return np.float32(loss)


# revision 10
# speedup vs baseline: 1.3769x; 1.3769x over previous
"""Distributed Trainium2 kernel for the DPCE loss.

loss = -mean_{b,p}[ sum_c dist_y[b,c,p] * logp[b,c,p] ]

where dist_y[:,0] = onehot0, dist_y[:,i>=1] = (z_i - mn_i)/(mx_i + eps - mn_i),
z_i = onehot_i * dist, mn/mx per (b, i) over all spatial positions, and
logp = log_softmax(net_output, axis=1).

Factorization (per batch b, class i in 1..3):
    sum_p zn_i * logp_i = (A_i - mn_i * L_i) / (mx_i + eps - mn_i)
with A_i = sum_p z_i * logp_i,  L_i = sum_p logp_i,  plus the class-0 term
E = sum_p onehot0 * logp_0.  All stats are single-pass masked reductions ->
fully data-parallel over the depth axis across 8 cores + tiny host combine.

mn_i = min_p z_i is exactly 0 unless EVERY position of batch b has class i
(z_i >= 0 with zeros wherever target != i); that never-in-practice case is
detected on the host (constant target slice) and resolved from f32 dist.

Engine split (measured rates drove this):
  ACT    exp x4, ln        (1x, ~1 elem/lane/cyc)
  DVE    lp/q/w via TT bf16 (2x mode), masks via TS bf16 (4x mode),
         mx via tensor_reduce (1x)
  GpSimd 3 of the elementwise adds/products (slow engine, takes a slice)
  PE     ALL add-reductions as ones-matmul accumulating into PSUM f32
"""

from contextlib import ExitStack

import numpy as np
import ml_dtypes

import concourse.bass as bass
import concourse.tile as tile
from concourse import bacc, mybir
from concourse.bass_utils import run_bass_kernel_spmd

# Problem shape (hardcoded per the task contract).
B, C, D, H, W = 2, 4, 128, 192, 192
NCORES = 8
DSH = D // NCORES            # depth slices per core
P = 128                      # SBUF partitions
SP = DSH * H * W             # spatial elems per (b, ch) per core = 589824
FTOT = SP // P               # free elems per partition = 4608
NCHUNK = 3
F = FTOT // NCHUNK           # chunk free size = 1536
NBLK = F // 512              # 512-wide blocks per chunk for PE sums = 3
EPS = 1e-8

# PSUM accumulators: one [1, 512] slot per (b, tgt), idx = b*NTGT + tgt,
# living at PSUM bank idx//4, partition quadrant (idx%4)*32 (PE output
# placement must be quadrant-aligned).  All NBLK blocks of all chunks
# accumulate into the same slot.
# tgt: 0..3 = L_c (sum lp_c), 4..6 = A_i (sum w_i*lp_i), 7 = E (sum c0*lp0)
NTGT = 8
NBANK = (B * NTGT + 2) // 3  # 6 (3 usable quadrant slots per PSUM bank)

# stats tile [P, NCOL] f32:
#   cols 0..NBANK-1 : drained per-bank PSUM sums (rows 0/32/64/96 meaningful)
#   cols NBANK + b*9 + ck*3 + (i-1) : per-partition mx_i of (b, chunk)
NCOL = 24

_BF = ml_dtypes.bfloat16

_compiled_nc = None


def _build():
    nc = bacc.Bacc("TRN2", target_bir_lowering=False, debug=False)
    bf = mybir.dt.bfloat16
    f32 = mybir.dt.float32
    AF = mybir.ActivationFunctionType
    Op = mybir.AluOpType

    x = nc.dram_tensor("x", [B, C, P, FTOT], bf, kind="ExternalInput").ap()
    t = nc.dram_tensor("t", [B, P, FTOT], bf, kind="ExternalInput").ap()
    d = nc.dram_tensor("d", [B, P, FTOT], bf, kind="ExternalInput").ap()
    out = nc.dram_tensor("out", [P, NCOL], f32, kind="ExternalOutput").ap()

    with tile.TileContext(nc) as tc, ExitStack() as ctx:
        inp = ctx.enter_context(tc.tile_pool(name="inp", bufs=2))
        work = ctx.enter_context(tc.tile_pool(name="work", bufs=2))
        singles = ctx.enter_context(tc.tile_pool(name="singles", bufs=1))
        psum = ctx.enter_context(tc.tile_pool(name="psum", bufs=1, space="PSUM"))

        stats = singles.tile([P, NCOL], f32)
        nc.vector.memset(stats[:], 0.0)
        ones = singles.tile([P, 1], bf)
        nc.vector.memset(ones[:], 1.0)
        accs = [psum.tile([P, 512], f32, name=f"acc{k}", tag=f"acc{k}") for k in range(NBANK)]
        for k in range(NBANK):
            nc.vector.memset(accs[k][:], 0.0)

        def pe_sum(src, b, tgt, ck):
            # accumulate the total sum of src into this (b, tgt) PSUM slot
            idx = b * NTGT + tgt
            bank, quad = divmod(idx, 3)
            dst = accs[bank][quad * 32 : quad * 32 + 1, :]
            for blk in range(NBLK):
                nc.tensor.matmul(
                    dst,
                    ones[:],
                    src[:, blk * 512 : (blk + 1) * 512],
                    start=(ck == 0 and blk == 0),
                    stop=(ck == NCHUNK - 1 and blk == NBLK - 1),
                )

        for b in range(B):
            for ck in range(NCHUNK):
                sl = slice(ck * F, (ck + 1) * F)
                xs = []
                for c in range(C):
                    xt = inp.tile([P, F], bf, tag=f"x{c}")
                    nc.sync.dma_start(xt[:], x[b, c, :, sl])
                    xs.append(xt)
                tt = inp.tile([P, F], bf, tag="t")
                nc.sync.dma_start(tt[:], t[b, :, sl])
                dd = inp.tile([P, F], bf, tag="d")
                nc.sync.dma_start(dd[:], d[b, :, sl])

                # lse = ln(sum_c exp(x_c)); |x| <= ~6 so no max-subtraction
                # is needed at f32 internal precision.
                es = []
                for c in range(C):
                    e = work.tile([P, F], bf, tag=f"e{c}")
                    nc.scalar.activation(e[:], xs[c][:], AF.Exp)
                    es.append(e)
                s01 = work.tile([P, F], bf, tag="s01")
                nc.gpsimd.tensor_tensor(s01[:], es[0][:], es[1][:], op=Op.add)
                s23 = work.tile([P, F], bf, tag="s23")
                nc.gpsimd.tensor_tensor(s23[:], es[2][:], es[3][:], op=Op.add)
                esum = work.tile([P, F], bf, tag="esum")
                nc.vector.tensor_tensor(esum[:], s01[:], s23[:], op=Op.add)
                lse = work.tile([P, F], bf, tag="lse")
                nc.scalar.activation(lse[:], esum[:], AF.Ln)

                # class masks (TS 4x) ; c0 kept for E
                c0 = work.tile([P, F], bf, tag="c0")
                nc.vector.tensor_scalar(c0[:], tt[:], 0.0, None, op0=Op.is_equal)

                for c in range(C):
                    # lp_c = x_c - lse ; L_c += PE-sum(lp_c)
                    lp = work.tile([P, F], bf, tag="lp")
                    nc.vector.tensor_tensor(lp[:], xs[c][:], lse[:], op=Op.subtract)
                    pe_sum(lp, b, c, ck)
                    q = work.tile([P, F], bf, tag="q")
                    if c == 0:
                        # E += PE-sum(c0 * lp0)
                        nc.vector.tensor_tensor(q[:], c0[:], lp[:], op=Op.mult)
                        pe_sum(q, b, 7, ck)
                    else:
                        cm = work.tile([P, F], bf, tag="cm")
                        nc.vector.tensor_scalar(
                            cm[:], tt[:], float(c), None, op0=Op.is_equal
                        )
                        # w_i = mask_i * dist (one of the three on GpSimd)
                        w = work.tile([P, F], bf, tag="w")
                        if c == 1:
                            nc.gpsimd.tensor_tensor(w[:], cm[:], dd[:], op=Op.mult)
                        else:
                            nc.vector.tensor_tensor(w[:], cm[:], dd[:], op=Op.mult)
                        nc.vector.tensor_reduce(
                            stats[:, mxcol(b, ck, c) : mxcol(b, ck, c) + 1],
                            w[:], axis=mybir.AxisListType.X, op=Op.max,
                        )
                        # A_i += PE-sum(w_i * lp_i)
                        nc.vector.tensor_tensor(q[:], w[:], lp[:], op=Op.mult)
                        pe_sum(q, b, 4 + (c - 1), ck)

        # drain PSUM accumulators: per-bank row-sums -> stats cols 0..NBANK-1
        for k in range(NBANK):
            nc.vector.tensor_reduce(
                stats[:, k : k + 1], accs[k][:, :], axis=mybir.AxisListType.X,
                op=Op.add,
            )
        nc.sync.dma_start(out[:], stats[:])

    nc.compile()
    return nc


def mxcol(b, ck, c):
    return NBANK + b * (NCHUNK * 3) + ck * 3 + (c - 1)


def _get_nc():
    global _compiled_nc
    if _compiled_nc is None:
        _compiled_nc = _build()
    return _compiled_nc


def kernel(net_output, target, dist):
    net_output = np.asarray(net_output, dtype=np.float32)
    target = np.asarray(target)
    dist = np.asarray(dist, dtype=np.float32)
    assert net_output.shape == (B, C, D, H, W)

    # host-side prep: bf16 casts + depth sharding
    xb = net_output.astype(_BF).reshape(B, C, NCORES, DSH * H * W)
    tb = target.reshape(B, D, H, W).astype(_BF).reshape(B, NCORES, DSH * H * W)
    db = dist.astype(_BF).reshape(B, NCORES, DSH * H * W)

    in_maps = []
    for r in range(NCORES):
        in_maps.append({
            "x": np.ascontiguousarray(xb[:, :, r]).reshape(B, C, P, FTOT),
            "t": np.ascontiguousarray(tb[:, r]).reshape(B, P, FTOT),
            "d": np.ascontiguousarray(db[:, r]).reshape(B, P, FTOT),
        })

    nc = _get_nc()
    res = run_bass_kernel_spmd(nc, in_maps, core_ids=list(range(NCORES)))

    # host combine (tiny: NCORES * 128 * 20 floats)
    L = np.zeros((B, C))
    A = np.zeros((B, C))
    E = np.zeros(B)
    mx = np.zeros((B, C))
    for r in range(NCORES):
        st = res.results[r]["out"].astype(np.float64)  # [P, NCOL]
        sums = np.array([
            st[(idx % 3) * 32, idx // 3] for idx in range(B * NTGT)
        ]).reshape(B, NTGT)
        for b in range(B):
            L[b] += sums[b, 0:4]
            A[b, 1:] += sums[b, 4:7]
            E[b] += sums[b, 7]
            for ck in range(NCHUNK):
                for i in range(1, C):
                    col = mxcol(b, ck, i)
                    mx[b, i] = max(mx[b, i], st[:, col].max())

    n_spatial = D * H * W
    total = 0.0
    for b in range(B):
        acc = E[b]
        tb_full = target.reshape(B, D, H, W)[b]
        const_cls = int(tb_full.flat[0]) if (tb_full == tb_full.flat[0]).all() else -1
        for i in range(1, C):
            # mn_i is exactly 0 unless the whole batch-b volume is class i
            mn = float(dist[b].min()) if const_cls == i else 0.0
            acc += (A[b, i] - mn * L[b, i]) / (mx[b, i] + EPS - mn)
        total += acc
    loss = -total / (B * n_spatial)
    return np.float32(loss)


# revision 11
# speedup vs baseline: 1.4418x; 1.0471x over previous
"""Distributed Trainium2 kernel for the DPCE loss.

loss = -mean_{b,p}[ sum_c dist_y[b,c,p] * logp[b,c,p] ]

where dist_y[:,0] = onehot0, dist_y[:,i>=1] = (z_i - mn_i)/(mx_i + eps - mn_i),
z_i = onehot_i * dist, mn/mx per (b, i) over all spatial positions, and
logp = log_softmax(net_output, axis=1).

Factorization (per batch b, class i in 1..3):
    sum_p zn_i * logp_i = (A_i - mn_i * L_i) / (mx_i + eps - mn_i)
with A_i = sum_p z_i * logp_i,  L_i = sum_p logp_i,  plus the class-0 term
E = sum_p onehot0 * logp_0.  All stats are single-pass masked reductions ->
fully data-parallel over the depth axis across 8 cores + tiny host combine.

mn_i = min_p z_i is exactly 0 unless EVERY position of batch b has class i
(z_i >= 0 with zeros wherever target != i); that never-in-practice case is
detected on the host (constant target slice) and resolved from f32 dist.

Engine split (measured rates drove this):
  ACT    exp, ln (1x rate)
  DVE    lp/q/w via TT bf16 (2x mode), masks via TS bf16 (4x), mx reduce (1x)
  GpSimd two esum adds + one w product (slow engine, takes a slice)
  PE     ALL add-reductions as ones-matmul accumulating into PSUM f32

Data layout: channels concatenated in the free dim -> [128, 4, F] tiles; one
DMA/ACT/TT instruction covers all four channels, with 0-stride broadcast APs
feeding lse / dist into per-channel ops.
"""

from contextlib import ExitStack

import numpy as np
import ml_dtypes

import concourse.bass as bass
import concourse.tile as tile
from concourse import bacc, mybir
from concourse.bass_utils import run_bass_kernel_spmd

# Problem shape (hardcoded per the task contract).
B, C, D, H, W = 2, 4, 128, 192, 192
NCORES = 8
DSH = D // NCORES            # depth slices per core
P = 128                      # SBUF partitions
SP = DSH * H * W             # spatial elems per (b, ch) per core = 589824
FTOT = SP // P               # free elems per partition = 4608
NCHUNK = 3
F = FTOT // NCHUNK           # chunk free size = 1536
NBLK = F // 512              # 512-wide blocks per class-chunk for PE sums = 3
EPS = 1e-8

# PSUM accumulators: one [1, 512] slot per (b, tgt), idx = b*NTGT + tgt, at
# PSUM bank idx//3, partition quadrant (idx%3)*32 (PE output placement must
# be quadrant-aligned and the AP encoding allows quadrants 0/32/64 only).
# tgt: 0..3 = L_c (sum lp_c), 4..6 = A_i (sum w_i*lp_i), 7 = E (sum c0*lp0)
NTGT = 8
NBANK = (B * NTGT + 2) // 3  # 6

# stats tile [P, NCOL] f32:
#   cols 0..NBANK-1 : drained per-bank PSUM sums (rows 0/32/64 meaningful)
#   cols NBANK + (b*NCHUNK + ck)*3 + (i-1) : per-partition mx_i of (b, chunk)
NCOL = 24

_BF = ml_dtypes.bfloat16

_compiled_nc = None


def mxcol(b, ck):
    return NBANK + (b * NCHUNK + ck) * 3


def _build():
    nc = bacc.Bacc("TRN2", target_bir_lowering=False, debug=False)
    bf = mybir.dt.bfloat16
    f32 = mybir.dt.float32
    AF = mybir.ActivationFunctionType
    Op = mybir.AluOpType

    x = nc.dram_tensor("x", [B, P, C, FTOT], bf, kind="ExternalInput").ap()
    t = nc.dram_tensor("t", [B, P, FTOT], bf, kind="ExternalInput").ap()
    d = nc.dram_tensor("d", [B, P, FTOT], bf, kind="ExternalInput").ap()
    out = nc.dram_tensor("out", [P, NCOL], f32, kind="ExternalOutput").ap()

    with tile.TileContext(nc) as tc, ExitStack() as ctx:
        inp = ctx.enter_context(tc.tile_pool(name="inp", bufs=2))
        work = ctx.enter_context(tc.tile_pool(name="work", bufs=2))
        singles = ctx.enter_context(tc.tile_pool(name="singles", bufs=1))
        psum = ctx.enter_context(tc.tile_pool(name="psum", bufs=1, space="PSUM"))

        stats = singles.tile([P, NCOL], f32)
        nc.vector.memset(stats[:], 0.0)
        ones = singles.tile([P, 1], bf)
        nc.vector.memset(ones[:], 1.0)
        accs = [
            psum.tile([P, 512], f32, name=f"acc{k}", tag=f"acc{k}")
            for k in range(NBANK)
        ]
        for k in range(NBANK):
            nc.vector.memset(accs[k][:], 0.0)

        def pe_sum(src_cls_ap, b, tgt, ck):
            # accumulate the total sum of a [P, F] class slice into (b, tgt)
            idx = b * NTGT + tgt
            bank, quad = divmod(idx, 3)
            dst = accs[bank][quad * 32 : quad * 32 + 1, :]
            for blk in range(NBLK):
                nc.tensor.matmul(
                    dst,
                    ones[:],
                    src_cls_ap[:, blk * 512 : (blk + 1) * 512],
                    start=(ck == 0 and blk == 0),
                    stop=(ck == NCHUNK - 1 and blk == NBLK - 1),
                )

        for b in range(B):
            for ck in range(NCHUNK):
                sl = slice(ck * F, (ck + 1) * F)
                xbig = inp.tile([P, C, F], bf, tag="xbig")
                nc.sync.dma_start(xbig[:], x[b, :, :, sl])
                tt = inp.tile([P, F], bf, tag="t")
                nc.sync.dma_start(tt[:], t[b, :, sl])
                dd = inp.tile([P, F], bf, tag="d")
                nc.sync.dma_start(dd[:], d[b, :, sl])

                # lse = ln(sum_c exp(x_c)); |x| <= ~6 so no max-subtraction
                # is needed at f32 internal precision.
                ebig = work.tile([P, C, F], bf, tag="ebig")
                nc.scalar.activation(ebig[:], xbig[:], AF.Exp)
                s01 = work.tile([P, F], bf, tag="s01")
                nc.gpsimd.tensor_tensor(s01[:], ebig[:, 0, :], ebig[:, 1, :], op=Op.add)
                s23 = work.tile([P, F], bf, tag="s23")
                nc.gpsimd.tensor_tensor(s23[:], ebig[:, 2, :], ebig[:, 3, :], op=Op.add)
                esum = work.tile([P, F], bf, tag="esum")
                nc.vector.tensor_tensor(esum[:], s01[:], s23[:], op=Op.add)
                lse = work.tile([P, F], bf, tag="lse")
                nc.scalar.activation(lse[:], esum[:], AF.Ln)

                # lp_c = x_c - lse for all c in one pass (lse broadcast)
                lpbig = work.tile([P, C, F], bf, tag="lpbig")
                nc.vector.tensor_tensor(
                    lpbig[:], xbig[:],
                    lse[:, None, :].broadcast_to([P, C, F]),
                    op=Op.subtract,
                )

                # masks: maskbig = [c0 | w1 | w2 | w3]
                maskbig = work.tile([P, C, F], bf, tag="maskbig")
                nc.vector.tensor_scalar(
                    maskbig[:, 0, :], tt[:], 0.0, None, op0=Op.is_equal
                )
                cbig = work.tile([P, C - 1, F], bf, tag="cbig")
                for i in range(1, C):
                    nc.vector.tensor_scalar(
                        cbig[:, i - 1, :], tt[:], float(i), None, op0=Op.is_equal
                    )
                # w_i = mask_i * dist (class 1 on GpSimd, classes 2-3 on DVE)
                nc.gpsimd.tensor_tensor(
                    maskbig[:, 1, :], cbig[:, 0, :], dd[:], op=Op.mult
                )
                nc.vector.tensor_tensor(
                    maskbig[:, 2:4, :], cbig[:, 1:3, :],
                    dd[:, None, :].broadcast_to([P, 2, F]),
                    op=Op.mult,
                )
                # per-chunk per-class max of w -> 3 stats columns at once
                nc.vector.tensor_reduce(
                    stats[:, mxcol(b, ck) : mxcol(b, ck) + 3],
                    maskbig[:, 1:4, :], axis=mybir.AxisListType.X, op=Op.max,
                )

                # q_c = mask_c * lp_c for all c in one pass
                qbig = work.tile([P, C, F], bf, tag="qbig")
                nc.vector.tensor_tensor(qbig[:], maskbig[:], lpbig[:], op=Op.mult)

                # PE sums: L_c from lpbig, E / A_i from qbig
                for c in range(C):
                    pe_sum(lpbig[:, c, :], b, c, ck)
                    pe_sum(qbig[:, c, :], b, 7 if c == 0 else 3 + c, ck)

        # drain PSUM accumulators: per-bank row-sums -> stats cols 0..NBANK-1
        for k in range(NBANK):
            nc.vector.tensor_reduce(
                stats[:, k : k + 1], accs[k][:, :], axis=mybir.AxisListType.X,
                op=Op.add,
            )
        nc.sync.dma_start(out[:], stats[:])

    nc.compile()
    return nc


def _get_nc():
    global _compiled_nc
    if _compiled_nc is None:
        _compiled_nc = _build()
    return _compiled_nc


def kernel(net_output, target, dist):
    net_output = np.asarray(net_output, dtype=np.float32)
    target = np.asarray(target)
    dist = np.asarray(dist, dtype=np.float32)
    assert net_output.shape == (B, C, D, H, W)

    # host-side prep: bf16 casts + depth sharding + channel-minor transpose
    xb = net_output.astype(_BF).reshape(B, C, NCORES, P, FTOT)
    tb = target.reshape(B, D, H, W).astype(_BF).reshape(B, NCORES, P, FTOT)
    db = dist.astype(_BF).reshape(B, NCORES, P, FTOT)

    in_maps = []
    for r in range(NCORES):
        in_maps.append({
            "x": np.ascontiguousarray(xb[:, :, r].transpose(0, 2, 1, 3)),
            "t": np.ascontiguousarray(tb[:, r]),
            "d": np.ascontiguousarray(db[:, r]),
        })

    nc = _get_nc()
    res = run_bass_kernel_spmd(nc, in_maps, core_ids=list(range(NCORES)))

    # host combine (tiny: NCORES * 128 * 24 floats)
    L = np.zeros((B, C))
    A = np.zeros((B, C))
    E = np.zeros(B)
    mx = np.zeros((B, C))
    for r in range(NCORES):
        st = res.results[r]["out"].astype(np.float64)  # [P, NCOL]
        sums = np.array([
            st[(idx % 3) * 32, idx // 3] for idx in range(B * NTGT)
        ]).reshape(B, NTGT)
        for b in range(B):
            L[b] += sums[b, 0:4]
            A[b, 1:] += sums[b, 4:7]
            E[b] += sums[b, 7]
            for ck in range(NCHUNK):
                col = mxcol(b, ck)
                mx[b, 1:] = np.maximum(mx[b, 1:], st[:, col : col + 3].max(axis=0))

    n_spatial = D * H * W
    total = 0.0
    for b in range(B):
        acc = E[b]
        tb_full = target.reshape(B, D, H, W)[b]
        const_cls = int(tb_full.flat[0]) if (tb_full == tb_full.flat[0]).all() else -1
        for i in range(1, C):
            # mn_i is exactly 0 unless the whole batch-b volume is class i
            mn = float(dist[b].min()) if const_cls == i else 0.0
            acc += (A[b, i] - mn * L[b, i]) / (mx[b, i] + EPS - mn)
        total += acc
    loss = -total / (B * n_spatial)
    return np.float32(loss)
